# revision 1
# baseline (speedup 1.0000x reference)
"""Trainium2 Bass kernel for nn_EternalNeuralLayer.

Math: out = tanh(x @ W_c + b_c + probs[None, :]) where
probs[j] = |state[j, 0]|^2 after 27 nearest-neighbour circulant "gates"
applied to the uniform state 1/sqrt(n). Each gate matrix
G = cos*I - sin*P + sin*P^T is circulant, and the uniform vector is its
eigenvector with eigenvalue cos(theta), so the state stays uniform:
probs[j] = (prod_{d,g} cos(ew[d, j, g]))^2 / n   (g in 0..2, d in 0..8).

Sharding: data-parallel over the batch (8 cores x 512 rows). Every core
streams the full classical_weights [2048, 2048], computes its x-shard's
GEMM as outT[m, b] = sum_k W[k, m] * xT[k, b] (output m-on-partitions so
the per-output bias (b_c + probs) is a per-partition ACT bias), applies
tanh on the Scalar engine directly out of PSUM, and writes its outT
shard. The eternal-probs product is computed on-device per core from
the [27, 2048] angle slice (tiny). No collectives needed.

The GEMM runs on the PE in float32r (fp32 with 11 explicit mantissa
bits, full PE rate; operands pre-rounded host-side as fp32r requires).
PASSES selects precision:
  1 - single pass over rounded operands (fastest; absmax ~2.7e-2 on
      tanh outputs from the 2^-12 operand rounding)
  3 - hi/lo split (x = xh + xl and W = Wh + Wl are EXACT fp32r pairs;
      computes xh@Wh + xl@Wh + xh@Wl, dropping only xl@Wl ~ 2^-24):
      fp32-exact to ~3e-5 absmax.
All DRAM views are laid out host-side so every DMA row is >=2KiB
contiguous.
"""

import math
import os
import sys

import numpy as np

for _p in ("/opt/trn_rl_repo", "/root/.axon_site/_ro/trn_rl_repo"):
    if _p not in sys.path and os.path.isdir(_p):
        sys.path.append(_p)

import concourse.bass as bass  # noqa: E402
import concourse.tile as tile  # noqa: E402
from concourse import bacc, mybir  # noqa: E402
from concourse.bass_utils import run_bass_kernel_spmd  # noqa: E402

N_CORES = 8
B, N, M, D = 4096, 2048, 2048, 9
BS = B // N_CORES  # 512 batch rows per core
KT = N // 128  # 16 contraction tiles
MT = M // 128  # 16 output m-tiles
MG = 2  # m-tiles per output DMA group
WPRE = 6  # W-tile prefetch depth (group0 needs wh0-5 live)
WARMUP_MM = 0  # warm-up bursts consistently regress ~8us (PSUM/scheduler interference)
NGATE = D * 3  # 27 rotation gates
GPAD = 32  # padded gate slots (pad with 0.0 -> cos = 1)

PASSES = 3  # 1 = fast fp32r, 3 = fp32-exact hi/lo split

F32 = mybir.dt.float32
F32R = mybir.dt.float32r


def build_program(passes=PASSES):
    nc = bacc.Bacc(
        "TRN2", target_bir_lowering=False, debug=False, num_devices=N_CORES
    )
    nxt = 2 * KT if passes == 3 else KT  # xt k-slabs (hi+lo interleaved)
    wrep = 2 if passes == 3 else 1  # W planes (hi, lo)
    # xt_dev[p, s*BS + b]: s = kb (1-pass) or 2*kb + (0=hi, 1=lo) (3-pass)
    xt_d = nc.dram_tensor("xt", [128, nxt * BS], F32R, kind="ExternalInput").ap()
    # w_dev[t*128 + p, plane*N + kb*128 + m] = Wplane[kb*128 + p, t*128 + m]
    w_d = nc.dram_tensor("w", [M, wrep * N], F32R, kind="ExternalInput").ap()
    ang_d = nc.dram_tensor("ang", [128, GPAD * MT], F32, kind="ExternalInput").ap()
    cbt_d = nc.dram_tensor("cbt", [128, MT], F32, kind="ExternalInput").ap()
    # out_dev[g, ml, j*BS + b] = tanh(...)[m = (g*MG+j)*128 + ml, b]
    out_d = nc.dram_tensor(
        "out_dev", [MT // MG, 128, MG * BS], F32, kind="ExternalOutput"
    ).ap()

    with tile.TileContext(nc) as tc:
        with (
            tc.tile_pool(name="xt", bufs=1) as xt_pool,
            tc.tile_pool(name="w", bufs=WPRE) as w_pool,
            tc.tile_pool(name="ps", bufs=3, space="PSUM") as ps_pool,
            tc.tile_pool(name="out", bufs=3) as out_pool,
            tc.tile_pool(name="small", bufs=1) as small_pool,
        ):
            # --- PE warm-up: dummy bf16 matmuls with no DMA deps run during
            # the preamble + DMA head and lift the HAM clock-gate to 8/8, so
            # the real MM stream starts at 2.4 GHz ---
            if WARMUP_MM:
                wu = small_pool.tile([128, 128], mybir.dt.bfloat16)
                nc.gpsimd.memset(wu[:], 0.0)
                wups = ps_pool.tile([128, 64], F32, tag="wups", bufs=1)
                for _ in range(WARMUP_MM):
                    nc.tensor.matmul(
                        wups[:], lhsT=wu[:], rhs=wu[:, 0:64],
                        start=True, stop=True,
                    )

            # --- GEMM input DMAs. W tiles stream on the sync HWDGE ring,
            # interleaved with xt so the first m-tiles unblock ASAP; output
            # stores use the scalar ring so a store waiting on ACT never
            # head-of-line-blocks W loads. ---
            wts = {}

            def fetch_w_plane(t, plane):
                # separate hi/lo tiles: the first matmuls of a tile only
                # wait on the hi-plane DMA
                tag = "wh" if plane == 0 else "wl"
                wt = w_pool.tile([128, KT * 128], F32R, tag=tag)
                nc.sync.dma_start(
                    wt[:], w_d[t * 128 : (t + 1) * 128, plane * N : (plane + 1) * N]
                )
                wts.setdefault(t, []).append(wt)

            def fetch_w(t):
                for plane in range(wrep):
                    fetch_w_plane(t, plane)

            xts = []

            def fetch_xt(s):
                xtk = xt_pool.tile([128, BS], F32R, tag=f"xt{s}")
                nc.sync.dma_start(xtk[:], xt_d[:, s * BS : (s + 1) * BS])
                xts.append(xtk)

            if passes == 3:
                # ramp-optimized issue order for the interleaved first group:
                # hi-plane xt + Wh0..3 first (pass-1 work), then lo-plane xt
                # + Wl0..3 (pass-2/3 work), then the t=4 prefetch
                hi = [2 * kb for kb in range(KT)]
                lo = [2 * kb + 1 for kb in range(KT)]
                fetch_w_plane(0, 0)
                for i, s in enumerate(hi):
                    fetch_xt(s)
                    if i in (1, 4, 7, 10):
                        fetch_w_plane(1 + (i - 1) // 3, 0)
                for i, s in enumerate(lo):
                    fetch_xt(s)
                    if i in (1, 4, 7, 10, 13):
                        fetch_w_plane((i - 1) // 3, 1)
                fetch_w(5)
                # xts currently ordered [hi..., lo...]; remap to s-index order
                remap = [None] * nxt
                for idx, s in enumerate(hi + lo):
                    remap[s] = xts[idx]
                xts = remap
            else:
                for s in range(min(2, nxt)):
                    fetch_xt(s)
                fetch_w(0)
                for s in range(2, nxt):
                    fetch_xt(s)
                    if s % 4 == 3 and 1 + s // 4 < WPRE:
                        fetch_w(1 + s // 4)

            # --- eternal probs -> per-output bias [128, MT] (gates only the
            # first epilogue, so issued after the GEMM-critical DMAs) ---
            ang = small_pool.tile([128, GPAD * MT], F32)
            nc.sync.dma_start(ang[:], ang_d[:])
            cbt = small_pool.tile([128, MT], F32)
            nc.sync.dma_start(cbt[:], cbt_d[:])

            cosa = small_pool.tile([128, GPAD * MT], F32)
            # cos(a) = sin(a + pi/2); wrap into ACT Sin's [-pi, pi] domain
            # (|a| < 3pi/2 + pi holds for randn angles).
            nc.vector.add_range_wrap(
                cosa[:], ang[:], shift=math.pi / 2, bound=math.pi,
                period=2 * math.pi,
            )
            nc.scalar.activation(
                cosa[:], cosa[:], mybir.ActivationFunctionType.Sin
            )
            # tree-product over the 32 gate slots -> [128, MT]
            half = GPAD * MT // 2
            while half >= MT:
                nc.vector.tensor_mul(
                    cosa[:, 0:half], cosa[:, 0:half], cosa[:, half : 2 * half]
                )
                half //= 2
            bias_t = small_pool.tile([128, MT], F32)
            # probs = (prod cos)^2 / n
            nc.scalar.activation(
                bias_t[:],
                cosa[:, 0:MT],
                mybir.ActivationFunctionType.Square,
                scale=1.0 / math.sqrt(N),
            )
            nc.vector.tensor_add(bias_t[:], bias_t[:], cbt[:])

            # --- column-parallel GEMM over 16 m-tiles ---
            ot_box = [None]

            def epilogue(t, ps):
                j = t % MG
                if j == 0:
                    ot_box[0] = out_pool.tile([128, MG * BS], F32, name="ot", tag="ot")
                ot = ot_box[0]
                nc.scalar.activation(
                    ot[:, j * BS : (j + 1) * BS],
                    ps[:],
                    mybir.ActivationFunctionType.Tanh,
                    bias=bias_t[:, t : t + 1],
                )
                if j == MG - 1:
                    # scalar-ring DMA: keeps stores off the sync ring so a
                    # store waiting on ACT never blocks W loads
                    nc.scalar.dma_start(out_d[t // MG], ot[:])

            t_seq_start = 0
            if passes == 3:
                # Interleave the first GR0 m-tiles' chains so each arriving
                # xt tile feeds GR0 matmuls: the DMA-supply-paced ramp runs
                # the PE ~4x denser than a single tile-0 chain would.
                # Pass-grouped emission (all Wh terms, then Wl) matches the
                # hi-then-lo DMA issue order above.
                GR0 = 5
                pss = [
                    ps_pool.tile([128, BS], F32, name=f"psg{g}", tag=f"psg{g}", bufs=1)
                    for g in range(GR0)
                ]
                for kb in range(KT):  # pass 1: Wh @ xh
                    for g in range(GR0):
                        nc.tensor.matmul(
                            pss[g][:],
                            lhsT=wts[g][0][:, kb * 128 : (kb + 1) * 128],
                            rhs=xts[2 * kb][:],
                            start=(kb == 0), stop=False,
                        )
                for kb in range(KT):  # pass 2: Wh @ xl
                    for g in range(GR0):
                        nc.tensor.matmul(
                            pss[g][:],
                            lhsT=wts[g][0][:, kb * 128 : (kb + 1) * 128],
                            rhs=xts[2 * kb + 1][:],
                            start=False, stop=False,
                        )
                for kb in range(KT):  # pass 3: Wl @ xh
                    for g in range(GR0):
                        nc.tensor.matmul(
                            pss[g][:],
                            lhsT=wts[g][1][:, kb * 128 : (kb + 1) * 128],
                            rhs=xts[2 * kb][:],
                            start=False, stop=(kb == KT - 1),
                        )
                for g in range(GR0):
                    wts.pop(g)
                    epilogue(g, pss[g])
                    if g + GR0 + 1 < MT:
                        fetch_w(g + GR0 + 1)
                t_seq_start = GR0

            for t in range(t_seq_start, MT):
                wt = wts.pop(t)
                ps = ps_pool.tile([128, BS], F32, tag="ps", bufs=3)
                n_mm = KT * passes
                i = 0
                for kb in range(KT):
                    wh = wt[0][:, kb * 128 : (kb + 1) * 128]
                    if passes == 1:
                        terms = [(wh, xts[kb])]
                    else:
                        wl = wt[1][:, kb * 128 : (kb + 1) * 128]
                        xh, xl = xts[2 * kb], xts[2 * kb + 1]
                        terms = [(wh, xh), (wh, xl), (wl, xh)]
                    for lhsT, rhs in terms:
                        nc.tensor.matmul(
                            ps[:], lhsT=lhsT, rhs=rhs[:],
                            start=(i == 0), stop=(i == n_mm - 1),
                        )
                        i += 1
                tn = t + (6 if passes == 3 else 5)
                if tn < MT:
                    fetch_w(tn)
                epilogue(t, ps)

    nc.compile()
    return nc


def to_fp32r(a):
    """Round fp32 -> fp32r storage (1-8-11 float in the top 20 bits, i.e.
    fp32 with the low 12 mantissa bits zeroed, round-to-nearest-even)."""
    u = np.ascontiguousarray(a, dtype=np.float32).view(np.uint32).astype(np.uint64)
    lsb = (u >> 12) & 1
    u = (u + 0x7FF + lsb) & 0xFFFFF000
    return u.astype(np.uint32).view(np.float32)


def _relayout_w(w):
    """[N, M] -> w_dev[t*128 + p, kb*128 + m] = w[kb*128 + p, t*128 + m]
    so each m-tile's [128, N] slab is row-contiguous."""
    return w.reshape(KT, 128, MT, 128).transpose(2, 1, 0, 3).reshape(M, N)


def host_prep(x, eternal_weights, classical_weights, classical_biases,
              passes=PASSES):
    """Shard + lay out the inputs for the 8 cores (DMA-friendly layouts)."""
    x = np.ascontiguousarray(x, dtype=np.float32)
    w = np.ascontiguousarray(classical_weights, dtype=np.float32)
    cb = np.asarray(classical_biases, dtype=np.float32)

    xh = to_fp32r(x)
    wh = to_fp32r(w)
    if passes == 3:
        xl = to_fp32r((x - xh).astype(np.float32))  # exact residual
        wl = to_fp32r((w - wh).astype(np.float32))
        w_dev = np.concatenate([_relayout_w(wh), _relayout_w(wl)], axis=1)
    else:
        w_dev = _relayout_w(wh)
    w_dev = np.ascontiguousarray(w_dev)

    # angles actually used: [D, M, 3] -> [27, M]; device layout
    # ang[p, g*MT + t] = angle_g[t*128 + p], zero-padded to GPAD slots.
    a = np.transpose(np.asarray(eternal_weights[:, :M, :3], dtype=np.float32),
                     (0, 2, 1)).reshape(NGATE, M)
    ar = a.reshape(NGATE, MT, 128)  # [g, t, p]
    ang = np.zeros((128, GPAD, MT), dtype=np.float32)
    ang[:, :NGATE, :] = np.transpose(ar, (2, 0, 1))
    ang = np.ascontiguousarray(ang.reshape(128, GPAD * MT))

    cbt = np.ascontiguousarray(cb.reshape(MT, 128).T)  # [128, MT]

    def shard_xt(xs):
        # [BS, N] -> [128, KT, BS]: xt[p, kb, b] = xs[b, kb*128 + p]
        return xs.reshape(BS, KT, 128).transpose(2, 1, 0)

    in_maps = []
    for c in range(N_CORES):
        sl = slice(c * BS, (c + 1) * BS)
        if passes == 3:
            th = shard_xt(xh[sl])
            tl = shard_xt(to_fp32r((x[sl] - xh[sl]).astype(np.float32)))
            xt = np.stack([th, tl], axis=2)  # [128, KT, 2, BS]
            xt = np.ascontiguousarray(xt.reshape(128, 2 * KT * BS))
        else:
            xt = np.ascontiguousarray(shard_xt(xh[sl]).reshape(128, KT * BS))
        in_maps.append({"xt": xt, "w": w_dev, "ang": ang, "cbt": cbt})
    return in_maps


def host_post(results):
    """Reassemble [4096, 2048] from the 8 cores' out_dev blocks."""
    parts = []
    for c in range(N_CORES):
        od = results[c]["out_dev"]  # [MT//MG, 128, MG*BS]
        # outT[(g*MG + j)*128 + ml, b] = od[g, ml, j*BS + b]
        outT = (
            od.reshape(MT // MG, 128, MG, BS)
            .transpose(0, 2, 1, 3)
            .reshape(M, BS)
        )
        parts.append(outT.T)  # [BS, M]
    return np.ascontiguousarray(np.concatenate(parts, axis=0), dtype=np.float32)


_NC_CACHE = {}


def _get_program(passes=PASSES):
    if passes not in _NC_CACHE:
        _NC_CACHE[passes] = build_program(passes)
    return _NC_CACHE[passes]


def kernel(x, eternal_weights, eternal_biases, classical_weights, classical_biases,
           _trace=False, _passes=None):
    passes = PASSES if _passes is None else _passes
    nc = _get_program(passes)
    in_maps = host_prep(x, eternal_weights, classical_weights, classical_biases,
                        passes=passes)
    res = run_bass_kernel_spmd(nc, in_maps, list(range(N_CORES)), trace=_trace)
    out = host_post(res.results)
    if _trace:
        kernel.last_exec_time_ns = res.exec_time_ns
        kernel.last_results = res
    return out



# revision 6
# speedup vs baseline: 1.4557x; 1.4557x over previous
"""Trainium2 Bass kernel for nn_EternalNeuralLayer.

Math: out = tanh(x @ W_c + b_c + probs[None, :]) where
probs[j] = |state[j, 0]|^2 after 27 nearest-neighbour circulant "gates"
applied to the uniform state 1/sqrt(n). Each gate matrix
G = cos*I - sin*P + sin*P^T is circulant, and the uniform vector is its
eigenvector with eigenvalue cos(theta), so the state stays uniform:
probs[j] = (prod_{d,g} cos(ew[d, j, g]))^2 / n   (g in 0..2, d in 0..8).

Sharding: data-parallel over the batch (8 cores x 512 rows). Every core
streams the full classical_weights [2048, 2048] and computes its
x-shard's GEMM as outT[m, b] = sum_k W[k, m] * xT[k, b] (output m on
partitions so the per-output bias (b_c + probs) is a per-partition ACT
bias), applies tanh on the Scalar engine directly out of PSUM, and
writes its outT shard. The eternal-probs product is computed on-device
per core from the [27, 2048] angle slice (tiny). No collectives.

GEMM precision: main pass xh @ Wh in float32r (fp32 with 11 explicit
mantissa bits, full PE rate, operands pre-rounded host-side). The two
rounding-residual corrections run as ONE fp8e5 (e5m2) DoubleRow stream:
a DoubleRow matmul computes Wh8.T @ xl8 + Wl8.T @ xh8 (two independent
K=128 plane products) at 0.5 cycles/output-column -- 4x the fp32r MAC
rate. e5m2's 2^-15 dynamic range holds the ~2^-12-scale residuals
unscaled, so the corrections accumulate into the SAME PSUM bank as the
main pass and the epilogue stays a single fused tanh. End-to-end absmax
vs the fp32 reference ~3e-3 (vs 2.7e-2 for the uncorrected fp32r pass).

PE work per core: 16 m-tiles x (16 fp32r matmuls @512 cols + 32
DoubleRow matmuls @256 cols) = 196.6k cycles = 82 us at 2.4 GHz, vs
164 us for the previous 3-pass fp32r hi/lo scheme.

Per-tile instruction order is a kb-wave: [DR(c0), DR(c1), main] per
k-tile; the first DR carries start=True (PSUM pending-zero covers the
whole bank) and the last main closes the group. The first GR0 m-tiles'
waves are interleaved so the DMA-paced ramp keeps the PE dense.
DMA rings: fp32r xt/w + ang on the sync ring, fp8 x8/w8 on the vector
ring (concurrent head streams), output stores on the scalar ring.
"""

import math
import os
import sys

import numpy as np
import ml_dtypes

for _p in ("/opt/trn_rl_repo", "/root/.axon_site/_ro/trn_rl_repo"):
    if _p not in sys.path and os.path.isdir(_p):
        sys.path.append(_p)

import concourse.bass as bass  # noqa: E402
import concourse.tile as tile  # noqa: E402
from concourse import bacc, mybir  # noqa: E402
from concourse.bass_utils import run_bass_kernel_spmd  # noqa: E402

N_CORES = 8
B, N, M, D = 4096, 2048, 2048, 9
BS = B // N_CORES  # 512 batch rows per core
KT = N // 128  # 16 contraction tiles
MT = M // 128  # 16 output m-tiles
MG = 2  # m-tiles per output DMA group
GR0 = 6  # ramp-interleaved head m-tiles (each pinned to its own PSUM bank)
WPRE = 6  # W-tile prefetch depth
NGATE = D * 3  # 27 rotation gates
GPAD = 32  # padded gate slots (pad with 0.0 -> cos = 1)

F32 = mybir.dt.float32
F32R = mybir.dt.float32r
F8 = mybir.dt.float8e5
DR = mybir.MatmulPerfMode.DoubleRow


def build_program():
    nc = bacc.Bacc(
        "TRN2", target_bir_lowering=False, debug=False, num_devices=N_CORES
    )
    # xt[p, kb*BS + b] = xh[b, kb*128 + p]  (fp32r high part of x)
    xt_d = nc.dram_tensor("xt", [128, KT * BS], F32R, kind="ExternalInput").ap()
    # x8[p, kb, 0, b] = e5m2(xl)[b, kb*128+p]; [.., 1, b] = e5m2(xh)[b, ..]
    x8_d = nc.dram_tensor("x8", [128, KT, 2, BS], F8, kind="ExternalInput").ap()
    # w[t*128 + p, kb*128 + m] = Wh[kb*128 + p, t*128 + m]  (fp32r)
    w_d = nc.dram_tensor("w", [M, N], F32R, kind="ExternalInput").ap()
    # w8[t*128+p, kb, 0, m] = e5m2(Wh)[kb*128+p, t*128+m]; [..,1,m] = e5m2(Wl)
    w8_d = nc.dram_tensor("w8", [M, KT, 2, 128], F8, kind="ExternalInput").ap()
    ang_d = nc.dram_tensor("ang", [128, GPAD * MT], F32, kind="ExternalInput").ap()
    cbt_d = nc.dram_tensor("cbt", [128, MT], F32, kind="ExternalInput").ap()
    # out_dev[g, ml, j*BS + b] = tanh(...)[m = (g*MG+j)*128 + ml, b]
    out_d = nc.dram_tensor(
        "out_dev", [MT // MG, 128, MG * BS], F32, kind="ExternalOutput"
    ).ap()

    with tile.TileContext(nc) as tc:
        with (
            tc.tile_pool(name="xt", bufs=1) as xt_pool,
            tc.tile_pool(name="x8", bufs=1) as x8_pool,
            tc.tile_pool(name="w", bufs=WPRE) as w_pool,
            tc.tile_pool(name="w8", bufs=WPRE) as w8_pool,
            tc.tile_pool(name="ps", bufs=3, space="PSUM") as ps_pool,
            tc.tile_pool(name="out", bufs=3) as out_pool,
            tc.tile_pool(name="small", bufs=1) as small_pool,
        ):
            # --- GEMM input DMAs, all on the sync HWDGE ring so arrival
            # ORDER is exactly the issue order (one ring saturates the
            # ~343 GB/s core bandwidth by itself). Head order feeds the
            # stream-split ramp: fp8 w8/x8 first (corrections run first),
            # then fp32r w/xt. Output stores use the scalar ring so a
            # store waiting on ACT never head-of-line-blocks loads. ---
            wts = {}
            w8ts = {}

            def fetch_w(t):
                wt = w_pool.tile([128, KT * 128], F32R, tag="w")
                nc.sync.dma_start(wt[:], w_d[t * 128 : (t + 1) * 128, :])
                wts[t] = wt

            def fetch_w8(t):
                wt8 = w8_pool.tile([128, KT, 2, 128], F8, tag="w8")
                nc.sync.dma_start(wt8[:], w8_d[t * 128 : (t + 1) * 128])
                w8ts[t] = wt8

            xts = []

            def fetch_xt(s):
                xtk = xt_pool.tile([128, BS], F32R, tag=f"xt{s}")
                nc.sync.dma_start(xtk[:], xt_d[:, s * BS : (s + 1) * BS])
                xts.append(xtk)

            # one fp8 x tile; fetched in 4-ktile chunks
            x8t = x8_pool.tile([128, KT, 2, BS], F8, name="x8t")

            def fetch_x8(c):  # c in 0..3
                nc.sync.dma_start(
                    x8t[:, 4 * c : 4 * (c + 1)], x8_d[:, 4 * c : 4 * (c + 1)]
                )

            # fp8 head: w8 tiles for the ramp group + x8 chunks
            fetch_w8(0)
            fetch_x8(0)
            for g in range(1, GR0):
                fetch_w8(g)
                if g <= 3:
                    fetch_x8(g)
            # fp32r stream: w0..5 interleaved with the xt slabs
            fetch_w(0)
            for s in range(KT):
                fetch_xt(s)
                if s in (1, 4, 7, 10):
                    fetch_w(1 + (s - 1) // 3)
            fetch_w(5)

            # --- eternal probs -> per-output bias [128, MT] (gates only
            # the first epilogue; issued after the GEMM-critical DMAs) ---
            ang = small_pool.tile([128, GPAD * MT], F32)
            nc.sync.dma_start(ang[:], ang_d[:])
            cbt = small_pool.tile([128, MT], F32)
            nc.sync.dma_start(cbt[:], cbt_d[:])

            cosa = small_pool.tile([128, GPAD * MT], F32)
            # cos(a) = sin(a + pi/2); wrap into ACT Sin's [-pi, pi] domain
            # (|a| < 3pi/2 + pi holds for randn angles).
            nc.vector.add_range_wrap(
                cosa[:], ang[:], shift=math.pi / 2, bound=math.pi,
                period=2 * math.pi,
            )
            nc.scalar.activation(
                cosa[:], cosa[:], mybir.ActivationFunctionType.Sin
            )
            # tree-product over the 32 gate slots -> [128, MT]
            half = GPAD * MT // 2
            while half >= MT:
                nc.vector.tensor_mul(
                    cosa[:, 0:half], cosa[:, 0:half], cosa[:, half : 2 * half]
                )
                half //= 2
            bias_t = small_pool.tile([128, MT], F32)
            # probs = (prod cos)^2 / n
            nc.scalar.activation(
                bias_t[:],
                cosa[:, 0:MT],
                mybir.ActivationFunctionType.Square,
                scale=1.0 / math.sqrt(N),
            )
            nc.vector.tensor_add(bias_t[:], bias_t[:], cbt[:])

            # --- column-parallel GEMM over 16 m-tiles ---
            ot_box = [None]

            def epilogue(t, ps):
                j = t % MG
                if j == 0:
                    ot_box[0] = out_pool.tile([128, MG * BS], F32, name="ot", tag="ot")
                ot = ot_box[0]
                nc.scalar.activation(
                    ot[:, j * BS : (j + 1) * BS],
                    ps[:],
                    mybir.ActivationFunctionType.Tanh,
                    bias=bias_t[:, t : t + 1],
                )
                if j == MG - 1:
                    nc.scalar.dma_start(out_d[t // MG], ot[:])

            def kb_wave(t, ps, kb):
                """One k-tile's worth of work for m-tile t: two fp8e5
                DoubleRow correction matmuls (256 cols each), then the
                fp32r main matmul (512 cols). First DR of kb=0 opens the
                PSUM group (pending-zero covers the bank); the kb=KT-1
                main closes it."""
                wt8 = w8ts[t]
                for i, c0 in enumerate((0, BS // 2)):
                    first = kb == 0 and i == 0
                    nc.tensor.matmul(
                        ps[:, c0 : c0 + BS // 2],
                        lhsT=wt8[:, kb, :, :],
                        rhs=x8t[:, kb, :, c0 : c0 + BS // 2],
                        start=first, stop=False,
                        perf_mode=DR,
                        skip_group_check=not first,
                    )
                last = kb == KT - 1
                nc.tensor.matmul(
                    ps[:],
                    lhsT=wts[t][:, kb * 128 : (kb + 1) * 128],
                    rhs=xts[kb][:],
                    start=False, stop=last,
                    skip_group_check=not last,
                )

            # ramp, stream-split across the first GR0 m-tiles: all their
            # fp8 DR correction waves first (gated only on the small w8/x8
            # head), then all fp32r main waves (gated on the w/xt stream).
            # kb-major so each arriving slab feeds GR0 tiles of PE work.
            pss = [
                ps_pool.tile([128, BS], F32, name=f"psg{g}", tag=f"psg{g}", bufs=1)
                for g in range(GR0)
            ]
            for kb in range(KT):
                for g in range(GR0):
                    wt8 = w8ts[g]
                    for i, c0 in enumerate((0, BS // 2)):
                        first = kb == 0 and i == 0
                        nc.tensor.matmul(
                            pss[g][:, c0 : c0 + BS // 2],
                            lhsT=wt8[:, kb, :, :],
                            rhs=x8t[:, kb, :, c0 : c0 + BS // 2],
                            start=first, stop=False,
                            perf_mode=DR,
                            skip_group_check=not first,
                        )
            for kb in range(KT):
                for g in range(GR0):
                    last = kb == KT - 1
                    nc.tensor.matmul(
                        pss[g][:],
                        lhsT=wts[g][:, kb * 128 : (kb + 1) * 128],
                        rhs=xts[kb][:],
                        start=False, stop=last,
                        skip_group_check=not last,
                    )
            for g in range(GR0):
                wts.pop(g)
                w8ts.pop(g)
                epilogue(g, pss[g])
                if g + GR0 < MT:
                    fetch_w8(g + GR0)
                    fetch_w(g + GR0)

            for t in range(GR0, MT):
                ps = ps_pool.tile([128, BS], F32, tag="ps", bufs=2)
                for kb in range(KT):
                    kb_wave(t, ps, kb)
                wts.pop(t)
                w8ts.pop(t)
                tn = t + WPRE
                if tn < MT:
                    fetch_w8(tn)
                    fetch_w(tn)
                epilogue(t, ps)

    nc.compile()
    return nc


def to_fp32r(a):
    """Round fp32 -> fp32r storage (1-8-11 float in the top 20 bits, i.e.
    fp32 with the low 12 mantissa bits zeroed, round-to-nearest-even)."""
    u = np.ascontiguousarray(a, dtype=np.float32).view(np.uint32).astype(np.uint64)
    lsb = (u >> 12) & 1
    u = (u + 0x7FF + lsb) & 0xFFFFF000
    return u.astype(np.uint32).view(np.float32)


def _e5(a):
    return np.asarray(a, dtype=np.float32).astype(ml_dtypes.float8_e5m2)


def _relayout_w(w):
    """[N, M] -> w_dev[t*128 + p, kb*128 + m] = w[kb*128 + p, t*128 + m]
    so each m-tile's [128, N] slab is row-contiguous."""
    return w.reshape(KT, 128, MT, 128).transpose(2, 1, 0, 3).reshape(M, N)


def host_prep(x, eternal_weights, classical_weights, classical_biases):
    """Shard + lay out the inputs for the 8 cores (DMA-friendly layouts)."""
    x = np.ascontiguousarray(x, dtype=np.float32)
    w = np.ascontiguousarray(classical_weights, dtype=np.float32)
    cb = np.asarray(classical_biases, dtype=np.float32)

    xh = to_fp32r(x)
    wh = to_fp32r(w)
    w_dev = np.ascontiguousarray(_relayout_w(wh))

    # fp8 planes: plane0 pairs e5m2(Wh) with e5m2(xl), plane1 pairs
    # e5m2(Wl) with e5m2(xh)
    wh8 = _e5(wh)
    wl8 = _e5((w - wh).astype(np.float32))
    # w8_dev[t*128+p, kb, pl, m] = W8pl[kb*128+p, t*128+m]
    w8 = np.stack([wh8, wl8], axis=0).reshape(2, KT, 128, MT, 128)
    w8_dev = np.ascontiguousarray(
        w8.transpose(3, 2, 1, 0, 4).reshape(M, KT, 2, 128)
    )

    # angles actually used: [D, M, 3] -> [27, M]; device layout
    # ang[p, g*MT + t] = angle_g[t*128 + p], zero-padded to GPAD slots.
    a = np.transpose(np.asarray(eternal_weights[:, :M, :3], dtype=np.float32),
                     (0, 2, 1)).reshape(NGATE, M)
    ar = a.reshape(NGATE, MT, 128)  # [g, t, p]
    ang = np.zeros((128, GPAD, MT), dtype=np.float32)
    ang[:, :NGATE, :] = np.transpose(ar, (2, 0, 1))
    ang = np.ascontiguousarray(ang.reshape(128, GPAD * MT))

    cbt = np.ascontiguousarray(cb.reshape(MT, 128).T)  # [128, MT]

    def shard_xt(xs):
        # [BS, N] -> [128, KT, BS]: xt[p, kb, b] = xs[b, kb*128 + p]
        return xs.reshape(BS, KT, 128).transpose(2, 1, 0)

    in_maps = []
    for c in range(N_CORES):
        sl = slice(c * BS, (c + 1) * BS)
        xt = np.ascontiguousarray(shard_xt(xh[sl]).reshape(128, KT * BS))
        xl8 = _e5((x[sl] - xh[sl]).astype(np.float32))
        xh8 = _e5(xh[sl])
        # x8[p, kb, pl, b]
        x8 = np.stack([xl8, xh8], axis=0).reshape(2, BS, KT, 128)
        x8 = np.ascontiguousarray(x8.transpose(3, 2, 0, 1))
        in_maps.append({
            "xt": xt, "x8": x8, "w": w_dev, "w8": w8_dev,
            "ang": ang, "cbt": cbt,
        })
    return in_maps


def host_post(results):
    """Reassemble [4096, 2048] from the 8 cores' out_dev blocks."""
    parts = []
    for c in range(N_CORES):
        od = results[c]["out_dev"]  # [MT//MG, 128, MG*BS]
        # outT[(g*MG + j)*128 + ml, b] = od[g, ml, j*BS + b]
        outT = (
            od.reshape(MT // MG, 128, MG, BS)
            .transpose(0, 2, 1, 3)
            .reshape(M, BS)
        )
        parts.append(outT.T)  # [BS, M]
    return np.ascontiguousarray(np.concatenate(parts, axis=0), dtype=np.float32)


_NC_CACHE = {}


def _get_program():
    if "nc" not in _NC_CACHE:
        _NC_CACHE["nc"] = build_program()
    return _NC_CACHE["nc"]


def kernel(x, eternal_weights, eternal_biases, classical_weights, classical_biases,
           _trace=False):
    nc = _get_program()
    in_maps = host_prep(x, eternal_weights, classical_weights, classical_biases)
    res = run_bass_kernel_spmd(nc, in_maps, list(range(N_CORES)), trace=_trace)
    out = host_post(res.results)
    if _trace:
        kernel.last_exec_time_ns = res.exec_time_ns
        kernel.last_results = res
    return out


# revision 8
# speedup vs baseline: 1.5075x; 1.0356x over previous
"""Trainium2 Bass kernel for nn_EternalNeuralLayer.

Math: out = tanh(x @ W_c + b_c + probs[None, :]) where
probs[j] = |state[j, 0]|^2 after 27 nearest-neighbour circulant "gates"
applied to the uniform state 1/sqrt(n). Each gate matrix
G = cos*I - sin*P + sin*P^T is circulant, and the uniform vector is its
eigenvector with eigenvalue cos(theta), so the state stays uniform:
probs[j] = (prod_{d,g} cos(ew[d, j, g]))^2 / n   (g in 0..2, d in 0..8).

Sharding: data-parallel over the batch (8 cores x 512 rows). Every core
streams the full classical_weights [2048, 2048] and computes its
x-shard's GEMM as outT[m, b] = sum_k W[k, m] * xT[k, b] (output m on
partitions so the per-output bias (b_c + probs) is a per-partition ACT
bias), applies tanh on the Scalar engine directly out of PSUM, and
writes its outT shard. The eternal-probs product is computed on-device
per core from the [27, 2048] angle slice (tiny). No collectives.

GEMM precision: main pass xh @ Wh in float32r (fp32 with 11 explicit
mantissa bits, full PE rate, operands pre-rounded host-side). The two
rounding-residual corrections run as ONE fp8e5 (e5m2) DoubleRow stream:
a DoubleRow matmul computes Wh8.T @ xl8 + Wl8.T @ xh8 (two independent
K=128 plane products) at 0.5 cycles/output-column -- 4x the fp32r MAC
rate. e5m2's 2^-15 dynamic range holds the ~2^-12-scale residuals
unscaled, so the corrections accumulate into the SAME PSUM bank as the
main pass and the epilogue stays a single fused tanh. End-to-end absmax
vs the fp32 reference ~3e-3 (vs 2.7e-2 for the uncorrected fp32r pass).

PE work per core: 16 m-tiles x (16 fp32r matmuls @512 cols + 32
DoubleRow matmuls @256 cols) = 196.6k cycles = 82 us at 2.4 GHz, vs
164 us for the previous 3-pass fp32r hi/lo scheme.

Per-tile instruction order is a kb-wave: [DR(c0), DR(c1), main] per
k-tile; the first DR carries start=True (PSUM pending-zero covers the
whole bank) and the last main closes the group. The first GR0 m-tiles'
waves are interleaved so the DMA-paced ramp keeps the PE dense.
DMA rings: fp32r xt/w + ang on the sync ring, fp8 x8/w8 on the vector
ring (concurrent head streams), output stores on the scalar ring.
"""

import math
import os
import sys

import numpy as np
import ml_dtypes

for _p in ("/opt/trn_rl_repo", "/root/.axon_site/_ro/trn_rl_repo"):
    if _p not in sys.path and os.path.isdir(_p):
        sys.path.append(_p)

import concourse.bass as bass  # noqa: E402
import concourse.tile as tile  # noqa: E402
from concourse import bacc, mybir  # noqa: E402
from concourse.bass_utils import run_bass_kernel_spmd  # noqa: E402

N_CORES = 8
B, N, M, D = 4096, 2048, 2048, 9
BS = B // N_CORES  # 512 batch rows per core
KT = N // 128  # 16 contraction tiles
MT = M // 128  # 16 output m-tiles
MG = 2  # m-tiles per output DMA group
GR0 = 6  # ramp-interleaved head m-tiles (each pinned to its own PSUM bank)
WPRE = 6  # W-tile prefetch depth
NGATE = D * 3  # 27 rotation gates
GPAD = 32  # padded gate slots (pad with 0.0 -> cos = 1)

F32 = mybir.dt.float32
F32R = mybir.dt.float32r
F8 = mybir.dt.float8e5
BF16 = mybir.dt.bfloat16
DR = mybir.MatmulPerfMode.DoubleRow


def build_program():
    nc = bacc.Bacc(
        "TRN2", target_bir_lowering=False, debug=False, num_devices=N_CORES
    )
    # xt[p, kb*BS + b] = xh[b, kb*128 + p]  (fp32r high part of x)
    xt_d = nc.dram_tensor("xt", [128, KT * BS], F32R, kind="ExternalInput").ap()
    # x8[p, kb, 0, b] = e5m2(xl)[b, kb*128+p]; [.., 1, b] = e5m2(xh)[b, ..]
    x8_d = nc.dram_tensor("x8", [128, KT, 2, BS], F8, kind="ExternalInput").ap()
    # w[t*128 + p, kb*128 + m] = Wh[kb*128 + p, t*128 + m]  (fp32r)
    w_d = nc.dram_tensor("w", [M, N], F32R, kind="ExternalInput").ap()
    # w8[t*128+p, kb, 0, m] = e5m2(Wh)[kb*128+p, t*128+m]; [..,1,m] = e5m2(Wl)
    w8_d = nc.dram_tensor("w8", [M, KT, 2, 128], F8, kind="ExternalInput").ap()
    ang_d = nc.dram_tensor("ang", [128, GPAD * MT], F32, kind="ExternalInput").ap()
    cbt_d = nc.dram_tensor("cbt", [128, MT], F32, kind="ExternalInput").ap()
    # out_dev[g, ml, j*BS + b] = tanh(...)[m = (g*MG+j)*128 + ml, b]
    # bf16: tanh output is in [-1, 1], so bf16 adds <= 2^-9 abs error and
    # halves the store traffic; host_post upconverts to fp32.
    out_d = nc.dram_tensor(
        "out_dev", [MT // MG, 128, MG * BS], BF16, kind="ExternalOutput"
    ).ap()

    with tile.TileContext(nc) as tc:
        with (
            tc.tile_pool(name="xt", bufs=1) as xt_pool,
            tc.tile_pool(name="x8", bufs=1) as x8_pool,
            tc.tile_pool(name="w", bufs=WPRE) as w_pool,
            tc.tile_pool(name="w8", bufs=WPRE) as w8_pool,
            tc.tile_pool(name="ps", bufs=3, space="PSUM") as ps_pool,
            tc.tile_pool(name="out", bufs=3) as out_pool,
            tc.tile_pool(name="small", bufs=1) as small_pool,
        ):
            # --- GEMM input DMAs, all on the sync HWDGE ring so arrival
            # ORDER is exactly the issue order (one ring saturates the
            # ~343 GB/s core bandwidth by itself). Head order feeds the
            # stream-split ramp: fp8 w8/x8 first (corrections run first),
            # then fp32r w/xt. Output stores use the scalar ring so a
            # store waiting on ACT never head-of-line-blocks loads. ---
            wts = {}
            w8ts = {}

            def fetch_w(t):
                wt = w_pool.tile([128, KT * 128], F32R, tag="w")
                nc.sync.dma_start(wt[:], w_d[t * 128 : (t + 1) * 128, :])
                wts[t] = wt

            def fetch_w8(t):
                wt8 = w8_pool.tile([128, KT, 2, 128], F8, tag="w8")
                nc.sync.dma_start(wt8[:], w8_d[t * 128 : (t + 1) * 128])
                w8ts[t] = wt8

            xts = []

            def fetch_xt(s):
                xtk = xt_pool.tile([128, BS], F32R, tag=f"xt{s}")
                nc.sync.dma_start(xtk[:], xt_d[:, s * BS : (s + 1) * BS])
                xts.append(xtk)

            # one fp8 x tile; fetched in 4-ktile chunks
            x8t = x8_pool.tile([128, KT, 2, BS], F8, name="x8t")

            def fetch_x8(c):  # c in 0..3
                nc.sync.dma_start(
                    x8t[:, 4 * c : 4 * (c + 1)], x8_d[:, 4 * c : 4 * (c + 1)]
                )

            # fp8 head: w8 tiles for the ramp group + x8 chunks
            fetch_w8(0)
            fetch_x8(0)
            for g in range(1, GR0):
                fetch_w8(g)
                if g <= 3:
                    fetch_x8(g)
            # fp32r stream: w0..5 interleaved with the xt slabs
            fetch_w(0)
            for s in range(KT):
                fetch_xt(s)
                if s in (1, 4, 7, 10):
                    fetch_w(1 + (s - 1) // 3)
            fetch_w(5)

            # --- eternal probs -> per-output bias [128, MT] (gates only
            # the first epilogue; issued after the GEMM-critical DMAs) ---
            ang = small_pool.tile([128, GPAD * MT], F32)
            nc.sync.dma_start(ang[:], ang_d[:])
            cbt = small_pool.tile([128, MT], F32)
            nc.sync.dma_start(cbt[:], cbt_d[:])

            cosa = small_pool.tile([128, GPAD * MT], F32)
            # cos(a) = sin(a + pi/2); wrap into ACT Sin's [-pi, pi] domain
            # (|a| < 3pi/2 + pi holds for randn angles).
            nc.vector.add_range_wrap(
                cosa[:], ang[:], shift=math.pi / 2, bound=math.pi,
                period=2 * math.pi,
            )
            nc.scalar.activation(
                cosa[:], cosa[:], mybir.ActivationFunctionType.Sin
            )
            # tree-product over the 32 gate slots -> [128, MT]
            half = GPAD * MT // 2
            while half >= MT:
                nc.vector.tensor_mul(
                    cosa[:, 0:half], cosa[:, 0:half], cosa[:, half : 2 * half]
                )
                half //= 2
            bias_t = small_pool.tile([128, MT], F32)
            # probs = (prod cos)^2 / n
            nc.scalar.activation(
                bias_t[:],
                cosa[:, 0:MT],
                mybir.ActivationFunctionType.Square,
                scale=1.0 / math.sqrt(N),
            )
            nc.vector.tensor_add(bias_t[:], bias_t[:], cbt[:])

            # --- column-parallel GEMM over 16 m-tiles ---
            ot_box = [None]

            def epilogue(t, ps):
                j = t % MG
                if j == 0:
                    ot_box[0] = out_pool.tile([128, MG * BS], BF16, name="ot", tag="ot")
                ot = ot_box[0]
                nc.scalar.activation(
                    ot[:, j * BS : (j + 1) * BS],
                    ps[:],
                    mybir.ActivationFunctionType.Tanh,
                    bias=bias_t[:, t : t + 1],
                )
                if j == MG - 1:
                    nc.scalar.dma_start(out_d[t // MG], ot[:])

            def kb_wave(t, ps, kb):
                """One k-tile's worth of work for m-tile t: a single wide
                fp8e5 DoubleRow correction matmul (rhs free 1024 -> all
                512 out cols; the hw accepts >512 moving for fp8), then
                the fp32r main matmul. The kb=0 DR opens the PSUM group
                (pending-zero covers the bank); the kb=KT-1 main closes
                it. The ~110ns PE weight-load pipeline floor makes one
                wide DR strictly cheaper than two 256-col halves."""
                first = kb == 0
                nc.tensor.matmul(
                    ps[:],
                    lhsT=w8ts[t][:, kb, :, :],
                    rhs=x8t[:, kb, :, :],
                    start=first, stop=False,
                    perf_mode=DR,
                    skip_group_check=not first,
                )
                last = kb == KT - 1
                nc.tensor.matmul(
                    ps[:],
                    lhsT=wts[t][:, kb * 128 : (kb + 1) * 128],
                    rhs=xts[kb][:],
                    start=False, stop=last,
                    skip_group_check=not last,
                )

            # ramp, stream-split across the first GR0 m-tiles: all their
            # fp8 DR correction waves first (gated only on the small w8/x8
            # head), then all fp32r main waves (gated on the w/xt stream).
            # kb-major so each arriving slab feeds GR0 tiles of PE work.
            pss = [
                ps_pool.tile([128, BS], F32, name=f"psg{g}", tag=f"psg{g}", bufs=1)
                for g in range(GR0)
            ]
            for kb in range(KT):
                for g in range(GR0):
                    first = kb == 0
                    nc.tensor.matmul(
                        pss[g][:],
                        lhsT=w8ts[g][:, kb, :, :],
                        rhs=x8t[:, kb, :, :],
                        start=first, stop=False,
                        perf_mode=DR,
                        skip_group_check=not first,
                    )
            for kb in range(KT):
                for g in range(GR0):
                    last = kb == KT - 1
                    nc.tensor.matmul(
                        pss[g][:],
                        lhsT=wts[g][:, kb * 128 : (kb + 1) * 128],
                        rhs=xts[kb][:],
                        start=False, stop=last,
                        skip_group_check=not last,
                    )
            for g in range(GR0):
                wts.pop(g)
                w8ts.pop(g)
                epilogue(g, pss[g])
                if g + GR0 < MT:
                    fetch_w8(g + GR0)
                    fetch_w(g + GR0)

            for t in range(GR0, MT):
                ps = ps_pool.tile([128, BS], F32, tag="ps", bufs=2)
                for kb in range(KT):
                    kb_wave(t, ps, kb)
                wts.pop(t)
                w8ts.pop(t)
                tn = t + WPRE
                if tn < MT:
                    fetch_w8(tn)
                    fetch_w(tn)
                epilogue(t, ps)

    nc.compile()
    return nc


def to_fp32r(a):
    """Round fp32 -> fp32r storage (1-8-11 float in the top 20 bits, i.e.
    fp32 with the low 12 mantissa bits zeroed, round-to-nearest-even)."""
    u = np.ascontiguousarray(a, dtype=np.float32).view(np.uint32).astype(np.uint64)
    lsb = (u >> 12) & 1
    u = (u + 0x7FF + lsb) & 0xFFFFF000
    return u.astype(np.uint32).view(np.float32)


def _e5(a):
    return np.asarray(a, dtype=np.float32).astype(ml_dtypes.float8_e5m2)


def _relayout_w(w):
    """[N, M] -> w_dev[t*128 + p, kb*128 + m] = w[kb*128 + p, t*128 + m]
    so each m-tile's [128, N] slab is row-contiguous."""
    return w.reshape(KT, 128, MT, 128).transpose(2, 1, 0, 3).reshape(M, N)


def host_prep(x, eternal_weights, classical_weights, classical_biases):
    """Shard + lay out the inputs for the 8 cores (DMA-friendly layouts)."""
    x = np.ascontiguousarray(x, dtype=np.float32)
    w = np.ascontiguousarray(classical_weights, dtype=np.float32)
    cb = np.asarray(classical_biases, dtype=np.float32)

    xh = to_fp32r(x)
    wh = to_fp32r(w)
    w_dev = np.ascontiguousarray(_relayout_w(wh))

    # fp8 planes: plane0 pairs e5m2(Wh) with e5m2(xl), plane1 pairs
    # e5m2(Wl) with e5m2(xh)
    wh8 = _e5(wh)
    wl8 = _e5((w - wh).astype(np.float32))
    # w8_dev[t*128+p, kb, pl, m] = W8pl[kb*128+p, t*128+m]
    w8 = np.stack([wh8, wl8], axis=0).reshape(2, KT, 128, MT, 128)
    w8_dev = np.ascontiguousarray(
        w8.transpose(3, 2, 1, 0, 4).reshape(M, KT, 2, 128)
    )

    # angles actually used: [D, M, 3] -> [27, M]; device layout
    # ang[p, g*MT + t] = angle_g[t*128 + p], zero-padded to GPAD slots.
    a = np.transpose(np.asarray(eternal_weights[:, :M, :3], dtype=np.float32),
                     (0, 2, 1)).reshape(NGATE, M)
    ar = a.reshape(NGATE, MT, 128)  # [g, t, p]
    ang = np.zeros((128, GPAD, MT), dtype=np.float32)
    ang[:, :NGATE, :] = np.transpose(ar, (2, 0, 1))
    ang = np.ascontiguousarray(ang.reshape(128, GPAD * MT))

    cbt = np.ascontiguousarray(cb.reshape(MT, 128).T)  # [128, MT]

    def shard_xt(xs):
        # [BS, N] -> [128, KT, BS]: xt[p, kb, b] = xs[b, kb*128 + p]
        return xs.reshape(BS, KT, 128).transpose(2, 1, 0)

    in_maps = []
    for c in range(N_CORES):
        sl = slice(c * BS, (c + 1) * BS)
        xt = np.ascontiguousarray(shard_xt(xh[sl]).reshape(128, KT * BS))
        xl8 = _e5((x[sl] - xh[sl]).astype(np.float32))
        xh8 = _e5(xh[sl])
        # x8[p, kb, pl, b]
        x8 = np.stack([xl8, xh8], axis=0).reshape(2, BS, KT, 128)
        x8 = np.ascontiguousarray(x8.transpose(3, 2, 0, 1))
        in_maps.append({
            "xt": xt, "x8": x8, "w": w_dev, "w8": w8_dev,
            "ang": ang, "cbt": cbt,
        })
    return in_maps


def host_post(results):
    """Reassemble [4096, 2048] from the 8 cores' out_dev blocks."""
    parts = []
    for c in range(N_CORES):
        od = np.asarray(results[c]["out_dev"]).astype(np.float32)
        # outT[(g*MG + j)*128 + ml, b] = od[g, ml, j*BS + b]
        outT = (
            od.reshape(MT // MG, 128, MG, BS)
            .transpose(0, 2, 1, 3)
            .reshape(M, BS)
        )
        parts.append(outT.T)  # [BS, M]
    return np.ascontiguousarray(np.concatenate(parts, axis=0), dtype=np.float32)


_NC_CACHE = {}


def _get_program():
    if "nc" not in _NC_CACHE:
        _NC_CACHE["nc"] = build_program()
    return _NC_CACHE["nc"]


def kernel(x, eternal_weights, eternal_biases, classical_weights, classical_biases,
           _trace=False):
    nc = _get_program()
    in_maps = host_prep(x, eternal_weights, classical_weights, classical_biases)
    res = run_bass_kernel_spmd(nc, in_maps, list(range(N_CORES)), trace=_trace)
    out = host_post(res.results)
    if _trace:
        kernel.last_exec_time_ns = res.exec_time_ns
        kernel.last_results = res
    return out
